# revision 1
# baseline (speedup 1.0000x reference)
"""Trainium2 Bass kernel for the KOSLM cell.

Computation (per reference):
    gates = x @ W_ih.T + b_ih + h_prev @ W_hh.T + b_hh          [B, 2H]
    z = tanh(gates[:, :H] + z_bias);  Tm = tanh(gates[:, H:] + m_bias)  (M_t = 1.5*Tm)
    innov = clip(z - 1.5*Tm*(A_base*c_prev), +-20)               A_base = -exp(log_lambda)
    hid = gelu(innov @ W1.T + b1)                                [B, 3H]
    Tk = tanh(hid @ W2.T + b2)                                   (K_t = 0.45*Tk)
    A_t = clip((1 - 0.675*Tk*Tm) * A_base, 0, 0.95)
    c_pre = A_t*c_prev + 0.45*Tk*z;  h_pre = 1.5*Tm*c_pre
    scale = min(100/sqrt(mean_H(c_pre^2) + 1e-6), 1)
    c_t = c_pre*scale;  h_t = h_pre*scale

Strategy:
  - Data parallel: batch (16384) sharded 2048 rows/core over 8 cores.
  - On device everything is computed in transposed layout [features, batch]
    so the weights are the stationary (lhsT) matmul operand: out[M=feat
    tile, N=batch cols] = W_chunk[K,M].T @ act[K, N].  Host pre-transposes
    activations/weights (free) and re-transposes outputs.
  - Matmul operands bf16 (PE full rate; fp32 matmul is 2x slower), PSUM
    accumulation fp32, elementwise tail fp32.
  - Per-core batch processed in 4 column-quarters of 512; activations for
    a quarter are SBUF-resident, weights are streamed (re-streamed per
    quarter, ~288 MiB/core, well under the PE roofline time).
  - Row RMS clamp needs a cross-partition (feature) reduction: done with a
    ones-vector matmul accumulating into PSUM over the 16 feature tiles,
    then a [1,128]-ones matmul broadcasts the per-column scale back to all
    128 partitions.
  - M_t clip(+-1.5) and B_t clip(+-0.95) are mathematical no-ops
    (M_t = 1.5*tanh in [-1.5,1.5], B_t = 0.45*tanh in [-0.45,0.45]) and
    are omitted.  A_t's clip is computed in full.
"""

import sys

sys.path.insert(0, "/opt/trn_rl_repo")

import numpy as np
import ml_dtypes

BF16NP = ml_dtypes.bfloat16

B, D, H = 16384, 1024, 2048
NCORES = 8
BS = B // NCORES  # 2048 batch rows per core
NQ = 4            # batch column quarters per core
NB = BS // NQ     # 512 batch cols per quarter
KG = (D + H) // 128  # 24 contraction chunks for the gates matmul
KH = H // 128        # 16 contraction chunks for W1
K3 = 3 * H // 128    # 48 contraction chunks for W2
MH = H // 128        # 16 feature tiles of H
M3 = 3 * H // 128    # 48 feature tiles of 3H

_CACHE = {}


def _build_program():
    import concourse.bacc as bacc
    import concourse.mybir as mybir
    import concourse.tile as tile

    AF = mybir.ActivationFunctionType
    ALU = mybir.AluOpType
    fp32 = mybir.dt.float32
    bf16 = mybir.dt.bfloat16

    nc = bacc.Bacc("TRN2", target_bir_lowering=False, debug=False,
                   num_devices=NCORES)

    with tile.TileContext(nc) as tc:
        with (
            tc.tile_pool(name="dram", bufs=1, space="DRAM") as dram,
            tc.tile_pool(name="consts", bufs=1) as consts,
            tc.tile_pool(name="xh_pool", bufs=1) as xh_pool,
            tc.tile_pool(name="iv_pool", bufs=1) as iv_pool,
            tc.tile_pool(name="hid_pool", bufs=1) as hid_pool,
            tc.tile_pool(name="zm_pool", bufs=1) as zm_pool,
            tc.tile_pool(name="w_pool", bufs=2) as w_pool,
            tc.tile_pool(name="cld_pool", bufs=2) as cld_pool,
            tc.tile_pool(name="tmp_pool", bufs=1) as tmp_pool,
            tc.tile_pool(name="g_pool", bufs=4) as g_pool,
            tc.tile_pool(name="ps_pool", bufs=4, space="PSUM") as ps_pool,
            tc.tile_pool(name="psn_pool", bufs=1, space="PSUM") as psn_pool,
            tc.tile_pool(name="psb_pool", bufs=1, space="PSUM") as psb_pool,
        ):
            # ---------------- DRAM I/O ----------------
            xh_d = dram.tile([128, NQ, KG, NB], bf16, kind="ExternalInput",
                             name="xh", uniquify=False)
            ct_d = dram.tile([MH, NQ, 128, NB], fp32, kind="ExternalInput",
                             name="ct_in", uniquify=False)
            wz_d = dram.tile([MH, 128, KG, 128], bf16, kind="ExternalInput",
                             name="wz", uniquify=False)
            wm_d = dram.tile([MH, 128, KG, 128], bf16, kind="ExternalInput",
                             name="wm", uniquify=False)
            w1_d = dram.tile([M3, 128, KH, 128], bf16, kind="ExternalInput",
                             name="w1", uniquify=False)
            w2_d = dram.tile([MH, 128, K3, 128], bf16, kind="ExternalInput",
                             name="w2", uniquify=False)
            zb_d = dram.tile([128, MH], fp32, kind="ExternalInput",
                             name="zb", uniquify=False)
            mb_d = dram.tile([128, MH], fp32, kind="ExternalInput",
                             name="mb", uniquify=False)
            ab_d = dram.tile([128, MH], fp32, kind="ExternalInput",
                             name="ab", uniquify=False)
            ab15_d = dram.tile([128, MH], fp32, kind="ExternalInput",
                               name="ab15", uniquify=False)
            b2_d = dram.tile([128, MH], fp32, kind="ExternalInput",
                             name="b2c", uniquify=False)
            b1_d = dram.tile([128, M3], fp32, kind="ExternalInput",
                             name="b1c", uniquify=False)
            cto_d = dram.tile([MH, NQ, 128, NB], fp32, kind="ExternalOutput",
                              name="ct_out", uniquify=False)
            hto_d = dram.tile([MH, NQ, 128, NB], fp32, kind="ExternalOutput",
                              name="ht_out", uniquify=False)
            cpre_d = dram.tile([MH, NQ, 128, NB], fp32, name="cpre_d")
            hpre_d = dram.tile([MH, NQ, 128, NB], fp32, name="hpre_d")

            # ---------------- constants to SBUF ----------------
            zb = consts.tile([128, MH], fp32, name="zb_sb")
            nc.sync.dma_start(out=zb, in_=zb_d)
            mb = consts.tile([128, MH], fp32, name="mb_sb")
            nc.sync.dma_start(out=mb, in_=mb_d)
            ab = consts.tile([128, MH], fp32, name="ab_sb")
            nc.sync.dma_start(out=ab, in_=ab_d)
            ab15 = consts.tile([128, MH], fp32, name="ab15_sb")
            nc.sync.dma_start(out=ab15, in_=ab15_d)
            b2c = consts.tile([128, MH], fp32, name="b2c_sb")
            nc.sync.dma_start(out=b2c, in_=b2_d)
            b1c = consts.tile([128, M3], fp32, name="b1c_sb")
            nc.sync.dma_start(out=b1c, in_=b1_d)
            ones_col = consts.tile([128, 1], bf16, name="ones_col")
            nc.vector.memset(ones_col, 1.0)
            ones_row = consts.tile([1, 128], bf16, name="ones_row")
            nc.vector.memset(ones_row, 1.0)
            eps_t = consts.tile([1, 1], fp32, name="eps_t")
            nc.vector.memset(eps_t, 1e-6)

            for q in range(NQ):
                xh_sb = xh_pool.tile([128, KG, NB], bf16, name="xh_sb",
                                     tag="xh")
                nc.sync.dma_start(out=xh_sb, in_=xh_d[:, q])
                z_sb = zm_pool.tile([128, MH, NB], bf16, name="z_sb", tag="z")
                m_sb = zm_pool.tile([128, MH, NB], bf16, name="m_sb", tag="m")
                iv_sb = iv_pool.tile([128, KH, NB], bf16, name="iv_sb",
                                     tag="iv")
                hid_sb = hid_pool.tile([128, K3, NB], bf16, name="hid_sb",
                                       tag="hid")

                # ---- Z phase: z = tanh(Wz.T @ xh + zb)
                for m in range(MH):
                    wt = w_pool.tile([128, KG, 128], bf16, name="wt", tag="w")
                    nc.sync.dma_start(out=wt, in_=wz_d[m])
                    ps = ps_pool.tile([128, NB], fp32, name="ps", tag="ps")
                    for k in range(KG):
                        nc.tensor.matmul(ps, wt[:, k], xh_sb[:, k],
                                         start=(k == 0), stop=(k == KG - 1))
                    nc.scalar.activation(z_sb[:, m], ps, AF.Tanh,
                                         bias=zb[:, m:m + 1])

                # ---- M phase: Tm = tanh(Wm.T @ xh + mb); innov fused
                for m in range(MH):
                    wt = w_pool.tile([128, KG, 128], bf16, name="wt", tag="w")
                    nc.sync.dma_start(out=wt, in_=wm_d[m])
                    ps = ps_pool.tile([128, NB], fp32, name="ps", tag="ps")
                    for k in range(KG):
                        nc.tensor.matmul(ps, wt[:, k], xh_sb[:, k],
                                         start=(k == 0), stop=(k == KG - 1))
                    nc.scalar.activation(m_sb[:, m], ps, AF.Tanh,
                                         bias=mb[:, m:m + 1])
                    cld = cld_pool.tile([128, NB], fp32, name="cld", tag="c")
                    nc.sync.dma_start(out=cld, in_=ct_d[m, q])
                    # t = (c * 1.5*A_base) * Tm
                    t1 = tmp_pool.tile([128, NB], fp32, name="t1", tag="t1")
                    nc.vector.scalar_tensor_tensor(
                        t1, cld, ab15[:, m:m + 1], m_sb[:, m],
                        op0=ALU.mult, op1=ALU.mult)
                    # iv = z - t ; clip to +-20, cast bf16
                    t2 = tmp_pool.tile([128, NB], fp32, name="t2", tag="t2")
                    nc.vector.scalar_tensor_tensor(
                        t2, t1, -1.0, z_sb[:, m], op0=ALU.mult, op1=ALU.add)
                    nc.vector.tensor_scalar(
                        iv_sb[:, m], t2, 20.0, -20.0,
                        op0=ALU.min, op1=ALU.max)

                # ---- D phase: hid = gelu(W1.T @ innov + b1)
                for j in range(M3):
                    wt1 = w_pool.tile([128, KH, 128], bf16, name="wt1",
                                      tag="w")
                    nc.sync.dma_start(out=wt1, in_=w1_d[j])
                    ps = ps_pool.tile([128, NB], fp32, name="ps", tag="ps")
                    for k in range(KH):
                        nc.tensor.matmul(ps, wt1[:, k], iv_sb[:, k],
                                         start=(k == 0), stop=(k == KH - 1))
                    nc.scalar.activation(hid_sb[:, j], ps, AF.Gelu,
                                         bias=b1c[:, j:j + 1])

                # ---- E phase: Tk = tanh(W2.T @ hid + b2); full gate tail
                psn = psn_pool.tile([1, NB], fp32, name="psn", tag="psn")
                for m in range(MH):
                    wt2 = w_pool.tile([128, K3, 128], bf16, name="wt2",
                                      tag="w")
                    nc.sync.dma_start(out=wt2, in_=w2_d[m])
                    ps = ps_pool.tile([128, NB], fp32, name="ps", tag="ps")
                    for k in range(K3):
                        nc.tensor.matmul(ps, wt2[:, k], hid_sb[:, k],
                                         start=(k == 0), stop=(k == K3 - 1))
                    tk = tmp_pool.tile([128, NB], fp32, name="tk", tag="tk")
                    nc.scalar.activation(tk, ps, AF.Tanh,
                                         bias=b2c[:, m:m + 1])
                    # u = (Tm * -0.675) * Tk     (= -K_t*M_t)
                    u = tmp_pool.tile([128, NB], fp32, name="u", tag="u")
                    nc.vector.scalar_tensor_tensor(
                        u, m_sb[:, m], -0.675, tk, op0=ALU.mult, op1=ALU.mult)
                    # a = clip((u + 1) * A_base, 0, 0.95)
                    a1 = tmp_pool.tile([128, NB], fp32, name="a1", tag="a1")
                    nc.vector.tensor_scalar(
                        a1, u, 1.0, ab[:, m:m + 1], op0=ALU.add, op1=ALU.mult)
                    a2 = tmp_pool.tile([128, NB], fp32, name="a2", tag="a2")
                    nc.vector.tensor_scalar(
                        a2, a1, 0.0, 0.95, op0=ALU.max, op1=ALU.min)
                    # w = (z * 0.45) * Tk        (= B_t*z_t)
                    wv = tmp_pool.tile([128, NB], fp32, name="wv", tag="wv")
                    nc.vector.scalar_tensor_tensor(
                        wv, z_sb[:, m], 0.45, tk, op0=ALU.mult, op1=ALU.mult)
                    cld = cld_pool.tile([128, NB], fp32, name="cld", tag="c")
                    nc.sync.dma_start(out=cld, in_=ct_d[m, q])
                    ac = tmp_pool.tile([128, NB], fp32, name="ac", tag="ac")
                    nc.vector.tensor_mul(ac, a2, cld)
                    cpre = tmp_pool.tile([128, NB], fp32, name="cpre",
                                         tag="cpre")
                    nc.vector.tensor_add(cpre, ac, wv)
                    hpre = tmp_pool.tile([128, NB], fp32, name="hpre",
                                         tag="hpre")
                    nc.vector.scalar_tensor_tensor(
                        hpre, m_sb[:, m], 1.5, cpre, op0=ALU.mult,
                        op1=ALU.mult)
                    sq = tmp_pool.tile([128, NB], bf16, name="sq", tag="sq")
                    nc.vector.tensor_mul(sq, cpre, cpre)
                    nc.tensor.matmul(psn, ones_col, sq, start=(m == 0),
                                     stop=(m == MH - 1),
                                     skip_group_check=True)
                    nc.sync.dma_start(out=cpre_d[m, q], in_=cpre)
                    nc.sync.dma_start(out=hpre_d[m, q], in_=hpre)

                # ---- G phase: RMS clamp scale + final outputs
                s_t = tmp_pool.tile([1, NB], fp32, name="s_t", tag="s_t")
                nc.scalar.activation(s_t, psn, AF.Sqrt,
                                     bias=eps_t[0:1, 0:1], scale=1.0 / H)
                r_t = tmp_pool.tile([1, NB], fp32, name="r_t", tag="r_t")
                nc.vector.reciprocal(r_t, s_t)
                sc_bf = tmp_pool.tile([1, NB], bf16, name="sc_bf",
                                      tag="sc_bf")
                nc.vector.tensor_scalar(
                    sc_bf, r_t, 100.0, 1.0, op0=ALU.mult, op1=ALU.min)
                psb = psb_pool.tile([128, NB], fp32, name="psb", tag="psb")
                nc.tensor.matmul(psb, ones_row, sc_bf, start=True, stop=True,
                                 skip_group_check=True)
                for m in range(MH):
                    cg = g_pool.tile([128, NB], fp32, name="cg", tag="g")
                    nc.sync.dma_start(out=cg, in_=cpre_d[m, q])
                    hg = g_pool.tile([128, NB], fp32, name="hg", tag="g")
                    nc.sync.dma_start(out=hg, in_=hpre_d[m, q])
                    cfin = g_pool.tile([128, NB], fp32, name="cfin", tag="go")
                    nc.vector.tensor_mul(cfin, cg, psb)
                    hfin = g_pool.tile([128, NB], fp32, name="hfin", tag="go")
                    nc.vector.tensor_mul(hfin, hg, psb)
                    nc.sync.dma_start(out=cto_d[m, q], in_=cfin)
                    nc.sync.dma_start(out=hto_d[m, q], in_=hfin)

    nc.compile()
    return nc


def _prep_shared(W_ih, b_ih, W_hh, b_hh, log_lambda, z_bias, m_bias,
                 W1, b1, W2, b2):
    """Host-side weight/constant layout prep (replicated to all cores)."""
    f32 = np.float32
    W_ih = np.asarray(W_ih, f32)
    W_hh = np.asarray(W_hh, f32)
    W1 = np.asarray(W1, f32)
    W2 = np.asarray(W2, f32)
    b_ih = np.asarray(b_ih, f32)
    b_hh = np.asarray(b_hh, f32)
    z_bias = np.asarray(z_bias, f32)
    m_bias = np.asarray(m_bias, f32)
    b1 = np.asarray(b1, f32)
    b2 = np.asarray(b2, f32)
    log_lambda = np.asarray(log_lambda, f32)

    # kxm weights: [K, M] with K = contraction
    Wz = np.concatenate([W_ih[:H].T, W_hh[:H].T], axis=0)    # [D+H, H]
    Wm = np.concatenate([W_ih[H:].T, W_hh[H:].T], axis=0)    # [D+H, H]
    W1T = W1.T                                               # [H, 3H]
    W2T = W2.T                                               # [3H, H]

    def wlay(Wkm, kchunks, mchunks):
        # [K, M] -> [mtile, p(=k within chunk), ktile, mcol]
        return np.ascontiguousarray(
            Wkm.reshape(kchunks, 128, mchunks, 128).transpose(2, 1, 0, 3)
        ).astype(BF16NP)

    A_base = -np.exp(log_lambda)                             # [H]
    zb_full = b_ih[:H] + b_hh[:H] + z_bias
    mb_full = b_ih[H:] + b_hh[H:] + m_bias

    def col128(v, nt):
        # [nt*128] -> [128, nt]
        return np.ascontiguousarray(v.reshape(nt, 128).T).astype(f32)

    return {
        "wz": wlay(Wz, KG, MH),
        "wm": wlay(Wm, KG, MH),
        "w1": wlay(W1T, KH, M3),
        "w2": wlay(W2T, K3, MH),
        "zb": col128(zb_full, MH),
        "mb": col128(mb_full, MH),
        "ab": col128(A_base, MH),
        "ab15": col128((1.5 * A_base).astype(f32), MH),
        "b2c": col128(b2, MH),
        "b1c": col128(b1, M3),
    }


def _prep_core(x_c, h_c, c_c):
    """Per-core activation layout prep."""
    XHT = np.concatenate([x_c.T, h_c.T], axis=0)             # [D+H, BS]
    xh = np.ascontiguousarray(
        XHT.reshape(KG, 128, NQ, NB).transpose(1, 2, 0, 3)
    ).astype(BF16NP)                                         # [128,NQ,KG,NB]
    ct = np.ascontiguousarray(
        c_c.T.reshape(MH, 128, NQ, NB).transpose(0, 2, 1, 3)
    ).astype(np.float32)                                     # [MH,NQ,128,NB]
    return xh, ct


def _run(inputs, trace=False):
    from concourse import bass_utils

    if "nc" not in _CACHE:
        _CACHE["nc"] = _build_program()
    nc = _CACHE["nc"]

    x = np.asarray(inputs["x"], np.float32)
    h_prev = np.asarray(inputs["h_prev"], np.float32)
    c_prev = np.asarray(inputs["c_prev"], np.float32)

    shared = _prep_shared(
        inputs["W_ih"], inputs["b_ih"], inputs["W_hh"], inputs["b_hh"],
        inputs["log_lambda"], inputs["z_bias"], inputs["m_bias"],
        inputs["W1"], inputs["b1"], inputs["W2"], inputs["b2"])

    in_maps = []
    for c in range(NCORES):
        sl = slice(c * BS, (c + 1) * BS)
        xh, ct = _prep_core(x[sl], h_prev[sl], c_prev[sl])
        m = dict(shared)
        m["xh"] = xh
        m["ct_in"] = ct
        in_maps.append(m)

    res = bass_utils.run_bass_kernel_spmd(
        nc, in_maps, list(range(NCORES)), trace=trace)

    h_parts, c_parts = [], []
    for c in range(NCORES):
        co = res.results[c]
        # [MH, NQ, 128, NB] -> [BS, H]: out[m,q,p,n] = val[NB*q+n, 128*m+p]
        c_parts.append(np.ascontiguousarray(
            co["ct_out"].transpose(1, 3, 0, 2).reshape(BS, H)))
        h_parts.append(np.ascontiguousarray(
            co["ht_out"].transpose(1, 3, 0, 2).reshape(BS, H)))
    h_t = np.concatenate(h_parts, axis=0)
    c_t = np.concatenate(c_parts, axis=0)
    return (h_t, c_t), res


def kernel(**inputs):
    (h_t, c_t), _ = _run(inputs, trace=False)
    return h_t, c_t
